# revision 39
# baseline (speedup 1.0000x reference)
"""Trainium2 Bass kernel for nn_BitLinear (LayerNorm -> 1.58-bit BitLinear).

Math notes
----------
Reference computes, per the module:
    xn    = LN(x) * ln_gamma + ln_beta            (eps = 1e-3)
    beta  = mean(|W|);  w_q = clip(round(W / (beta + 1e-5)), -1, 1)
    gamma = max(|xn|)   (global absmax)
    xq    = clip(xn * 128 / gamma, -128 + 1e-5, 128 - 1e-5)
    y     = (xq @ w_q) * (gamma * beta / 128)

The gamma factor cancels exactly: (xn*128/gamma) @ w_q * (gamma*beta/128)
== (xn @ w_q) * beta.  The clip only affects elements within relative
7.8e-8 of the global absmax -- far below f32 matmul roundoff.  So the
kernel computes y = (LN(x) @ w_q) * beta, fully data-parallel over
tokens (no collectives).

LayerNorm folds into the matmul:
    LN(x) @ wq = s * (x @ wq - mu * colsum),   colsum[u] = sum_d wq[d,u]
The PE runs on RAW x shipped pre-transposed from the host (no on-device
transposes, no normalize pass).  The -mu*colsum term is PRELOADED into
PSUM before each tile's matmuls: the matmuls run with start=False and
accumulate on top (a one-time prologue "warmup" matmul per PSUM slot
sets the has_written bits so accumulate mode stays armed; engine writes
overwrite values but don't clear the bits).  The epilogue is a single
per-partition scale y = ps * (s*beta), alternating DVE/ACT per tile.

Precision/throughput split (measured on HW: bf16 K=128 matmul ~230ns,
fp8 DoubleRow K=256 matmul ~259ns -- 1.97x per unit contraction):
4 of 8 k-blocks run in bf16, the other 4 as two fp8(e4m3) DoubleRow
pairs per 512-wide half.  The fp8 quantization noise on half the
contraction costs rel-err 1.888e-2 vs the 2e-2 gate (measured on HW,
bit-matching a numpy simulation of the same scheme; the statistic is
an average over 33M outputs, so it is insensitive to the input draw).
The ternary w_q is exact in both dtypes.  Stats (mean/var) come from a
bf16 row-layout copy of x.

Host prep (one-time, tiny vs the 128 MB activation tensor): ternarize W
(beta = mean|W| "computed once" per the sharding hint), colsum, dtype
casts + transpose.  All O(tokens) math stays on device.

Sharding: data-parallel over the 32768 tokens, 4096 per core; weight
replicated.  If ln_gamma/ln_beta are non-trivial (not the case for the
reference inputs), a slower all-bf16 variant of the same program folds
gamma into the weights and adds beta*(ln_beta @ w_q) in the epilogue.

Scheduling notes (from perfetto traces): DMA rings are per-engine FIFO
and a dma_start can block its issuing engine's queue until ring slots
free, so the scalar (ACT) engine -- which runs sqrt/preload/epilogues
-- only issues DMAs whose data is needed late; a dummy sqrt leads the
scalar queue to hoist the one-time 1.3us ACT_TABLE_LOAD off the
critical path; mid-kernel y drains ride the otherwise-idle gpsimd
(SWDGE) ring; the PSUM-preload chain (stats -> -mu -> preload) never
depends on the sqrt path so PSUM recycling can't stall on the ACT
queue; the first four preloads run on DVE while the scalar queue is
still draining prologue dma_starts.
"""

import numpy as np
import ml_dtypes

B, S, D, U = 4, 8192, 1024, 1024
N_CORES = 8
TOK = (B * S) // N_CORES  # 4096 tokens per core
P = 128
KB = D // P               # 8 contraction blocks
KBF = 4                   # k-blocks in bf16; the last KB-KBF run in fp8
NTILES = TOK // P         # 32 token tiles per core
GT = 8                    # token tiles per DMA group
NG = NTILES // GT         # 4 groups
LOOK = 3                  # front-runs stats/preload this many tiles ahead
NPS = 4                   # PSUM slots (2 banks each)
LN_EPS = 1e-3
EPS = 1e-5

BF16 = ml_dtypes.bfloat16
FP8 = ml_dtypes.float8_e4m3fn

_NC_CACHE = {}


def _build_mixed(kbf=KBF, apply_beta=False):
    """bf16 + fp8-DoubleRow kernel (kbf=8 => all-bf16 general variant)."""
    import concourse.bacc as bacc
    import concourse.mybir as mybir
    import concourse.tile as tile
    from concourse.bass import ts

    fp32 = mybir.dt.float32
    bf16 = mybir.dt.bfloat16
    fp8 = mybir.dt.float8e4
    AF = mybir.ActivationFunctionType
    OP = mybir.AluOpType
    DR = mybir.MatmulPerfMode.DoubleRow
    kb8 = KB - kbf

    nc = bacc.Bacc()
    xb_h = nc.dram_tensor("xbt", [kbf * P, TOK], bf16, kind="ExternalInput")
    x8_h = (
        nc.dram_tensor("x8t", [kb8 * P, TOK], fp8, kind="ExternalInput")
        if kb8
        else None
    )
    xr_h = nc.dram_tensor("xr", [TOK, D], bf16, kind="ExternalInput")
    wb_h = nc.dram_tensor("wqb", [kbf * P, U], bf16, kind="ExternalInput")
    w8_h = (
        nc.dram_tensor("wq8", [kb8 * P, U], fp8, kind="ExternalInput")
        if kb8
        else None
    )
    cs_h = nc.dram_tensor("cs", [P, U], bf16, kind="ExternalInput")
    bc_h = nc.dram_tensor("bcol", [P, 1], fp32, kind="ExternalInput")
    rb_h = (
        nc.dram_tensor("rb", [P, U], fp32, kind="ExternalInput")
        if apply_beta
        else None
    )
    y_h = nc.dram_tensor("y", [TOK, U], bf16, kind="ExternalOutput")

    xr_view = xr_h[:, :].rearrange("(o p) d -> p o d", p=P)    # [128, 32, 1024]
    xb_view = xb_h[:, :].rearrange("(k q) t -> q k t", q=P)    # [128, kbf, 4096]
    x8_view = x8_h[:, :].rearrange("(k q) t -> q k t", q=P) if kb8 else None
    wb_view = wb_h[:, :].rearrange("(k q) u -> q k u", q=P)    # [128, kbf, 1024]
    w8_view = w8_h[:, :].rearrange("(k q) u -> q k u", q=P) if kb8 else None
    y_view = y_h[:, :].rearrange("(o p) u -> p o u", p=P)      # [128, 32, 1024]

    with tile.TileContext(nc) as tc:
        with (
            tc.tile_pool(name="singles", bufs=1) as singles,
            tc.tile_pool(name="xrg", bufs=4) as xrg_pool,
            tc.tile_pool(name="xbg", bufs=3) as xbg_pool,
            tc.tile_pool(name="x8g", bufs=3) as x8g_pool,
            tc.tile_pool(name="yg", bufs=2) as yg_pool,
            tc.tile_pool(name="stats", bufs=2 + LOOK) as stats_pool,
            tc.tile_pool(name="ps_y", bufs=NPS, space="PSUM") as ps_pool,
        ):
            xr_tiles = [None] * NG
            xb_tiles = [None] * NG
            x8_tiles = [None] * NG
            y_tiles = [None] * NG

            def issue_xg(g):
                tb = xbg_pool.tile([P, kbf, GT * P], bf16, tag="xb", name=f"xb{g}")
                base = g * GT * P
                for c in range(2):
                    nc.sync.dma_start(
                        out=tb[:, :, c * 512 : (c + 1) * 512],
                        in_=xb_view[:, :, base + c * 512 : base + (c + 1) * 512],
                    )
                xb_tiles[g] = tb
                if kb8:
                    t8 = x8g_pool.tile(
                        [P, kb8, GT * P], fp8, tag="x8", name=f"x8{g}"
                    )
                    nc.sync.dma_start(
                        out=t8, in_=x8_view[:, :, base : base + GT * P]
                    )
                    x8_tiles[g] = t8

            def issue_xr(g):
                t = xrg_pool.tile([P, GT, D], bf16, tag="xr", name=f"xr{g}")
                nc.scalar.dma_start(out=t, in_=xr_view[:, g * GT : (g + 1) * GT, :])
                xr_tiles[g] = t

            # ---- prologue: group-0 + weights arrive in dependency order ----
            # tile-0 critical chain: {cs,bc,xr c0} -> stats -> preload, plus
            # {xb0/x80 first chunk, wb k0} -> first matmuls; everything else
            # streams behind in per-kb / per-2-tile chunks.
            # Dummy sqrt FIRST on the scalar queue: pulls the 1.3us
            # ACT_TABLE_LOAD to t~3us, before any dma_start can block it.
            warm_s = singles.tile([P, 1], fp32)
            nc.vector.memset(warm_s, 1.0)
            nc.scalar.activation(
                out=warm_s, in_=warm_s, func=AF.Sqrt, bias=0.0, scale=1.0
            )
            wb_sb = singles.tile([P, kbf, U], bf16)
            w8_sb = singles.tile([P, kb8, U], fp8, name="w8_sb") if kb8 else None
            cs_sb = singles.tile([P, U], bf16)
            nc.sync.dma_start(out=cs_sb, in_=cs_h[:, :])
            bc_sb = singles.tile([P, 1], fp32)
            nc.sync.dma_start(out=bc_sb, in_=bc_h[:, :])
            if apply_beta:
                rb_sb = singles.tile([P, U], fp32)
                nc.sync.dma_start(out=rb_sb, in_=rb_h[:, :])
            xr0 = xrg_pool.tile([P, GT, D], bf16, tag="xr", name="xr0")
            xr_tiles[0] = xr0
            nc.scalar.dma_start(out=xr0[:, 0:1, :], in_=xr_view[:, 0:1, :])
            nc.scalar.dma_start(out=xr0[:, 1:2, :], in_=xr_view[:, 1:2, :])
            xb0 = xbg_pool.tile([P, kbf, GT * P], bf16, tag="xb", name="xb0")
            xb_tiles[0] = xb0
            nc.sync.dma_start(out=xb0[:, :, 0:256], in_=xb_view[:, :, 0:256])
            if kb8:
                x80 = x8g_pool.tile([P, kb8, GT * P], fp8, tag="x8", name="x80")
                x8_tiles[0] = x80
                nc.sync.dma_start(out=x80[:, :, 0:256], in_=x8_view[:, :, 0:256])
            nc.scalar.dma_start(out=wb_sb[:, 0, :], in_=wb_view[:, 0, :])
            nc.scalar.dma_start(out=wb_sb[:, 1, :], in_=wb_view[:, 1, :])
            nc.scalar.dma_start(out=xr0[:, 2:4, :], in_=xr_view[:, 2:4, :])
            nc.sync.dma_start(out=wb_sb[:, 2:kbf, :], in_=wb_view[:, 2:kbf, :])
            if kb8:
                nc.sync.dma_start(out=w8_sb, in_=w8_view[:, :, :])
            nc.sync.dma_start(out=xb0[:, :, 256:512], in_=xb_view[:, :, 256:512])
            if kb8:
                nc.sync.dma_start(
                    out=x80[:, :, 256:512], in_=x8_view[:, :, 256:512]
                )
            nc.scalar.dma_start(out=xr0[:, 4:GT, :], in_=xr_view[:, 4:GT, :])
            nc.sync.dma_start(out=xb0[:, :, 512:768], in_=xb_view[:, :, 512:768])
            if kb8:
                nc.sync.dma_start(
                    out=x80[:, :, 512:1024], in_=x8_view[:, :, 512:1024]
                )
            nc.sync.dma_start(out=xb0[:, :, 768:1024], in_=xb_view[:, :, 768:1024])

            eps_t = singles.tile([P, 1], fp32)
            nc.vector.memset(eps_t, LN_EPS)


            # ---- per-tile pieces ----
            def front(i):
                """Stats chain + PSUM preload; runs LOOK tiles ahead of PE.

                The preload chain (aggr -> nm -> preload) never touches the
                sqrt path, so a slow ACT queue can't stall PSUM recycling;
                recip/a (epilogue scale) are deferred to back().
                """
                g, il = divmod(i, GT)
                xv = xr_tiles[g][:, il, :]
                xvr = xv.rearrange("p (n f) -> p n f", f=512)
                st = stats_pool.tile([P, 2, 6], fp32, tag="st")
                nc.vector.bn_stats(out=st[:, 0, :], in_=xvr[:, 0, :])
                nc.vector.bn_stats(out=st[:, 1, :], in_=xvr[:, 1, :])
                mv = stats_pool.tile([P, 2], fp32, tag="mv")
                nc.vector.bn_aggr(out=mv, in_=st)
                nm = stats_pool.tile([P, 1], fp32, tag="nm")
                nc.vector.tensor_scalar(
                    out=nm, in0=mv[:, 0:1], scalar1=-1.0, scalar2=None, op0=OP.mult
                )
                # preload: ps <- cs * (-mu)   (overwrite; has_written bits
                # stay set).  Tiles 0-3 use DVE: the scalar queue is still
                # draining its prologue dma_starts at that point.
                ps = ps_pool.tile([P, U], fp32, tag="ps")
                if i < 4 and not apply_beta:
                    pass  # start=True matmuls; correction lands in epilogue
                elif i < 6:
                    nc.vector.tensor_scalar(
                        out=ps, in0=cs_sb, scalar1=nm, scalar2=None, op0=OP.mult
                    )
                else:
                    nc.scalar.mul(out=ps, in_=cs_sb, mul=nm)
                sq = stats_pool.tile([P, 1], fp32, tag="sq")
                nc.scalar.activation(
                    out=sq, in_=mv[:, 1:2], func=AF.Sqrt, bias=eps_t, scale=1.0
                )
                return sq, nm, ps

            def back(i, sq, nm, ps):
                """bf16 + fp8-DoubleRow matmul sweep + scale epilogue."""
                g, il = divmod(i, GT)
                s_t = stats_pool.tile([P, 1], fp32, tag="s")
                nc.vector.reciprocal(s_t, sq)
                a_t = stats_pool.tile([P, 1], fp32, tag="a")
                nc.vector.tensor_tensor(a_t, s_t, bc_sb, OP.mult)
                if il == 0:
                    y_tiles[g] = yg_pool.tile([P, GT, U], bf16, tag="y", name=f"y{g}")
                lbt, l8 = xb_tiles[g], x8_tiles[g]
                tok = slice(il * P, (il + 1) * P)
                npair = kb8 // 2
                for h in range(2):
                    for k in range(kbf):
                        nc.tensor.matmul(
                            ps[:, ts(h, 512)], lhsT=lbt[:, k, tok],
                            rhs=wb_sb[:, k, ts(h, 512)],
                            start=(i < 4 and not apply_beta and k == 0),
                            stop=(npair == 0 and k == kbf - 1),
                        )
                    for j in range(npair):
                        pr = slice(2 * j, 2 * j + 2)
                        nc.tensor.matmul(
                            ps[:, ts(h, 512)], lhsT=l8[:, pr, tok],
                            rhs=w8_sb[:, pr, ts(h, 512)],
                            start=False, stop=(j == npair - 1), perf_mode=DR,
                        )
                # epilogue: y = ps * a (+ rb); alternate engines in the fast
                # path so neither queue saturates
                yv = y_tiles[g][:, il, :]
                if i < 4 and not apply_beta:
                    # no preload happened: y = a*ps + cs*(nm*a)
                    na = stats_pool.tile([P, 1], fp32, tag="na")
                    nc.vector.tensor_tensor(na, nm, a_t, OP.mult)
                    t4 = stats_pool.tile([P, U], fp32, tag="t4")
                    nc.scalar.mul(out=t4, in_=cs_sb, mul=na)
                    nc.vector.scalar_tensor_tensor(
                        out=yv, in0=ps, scalar=a_t, in1=t4,
                        op0=OP.mult, op1=OP.add,
                    )
                elif apply_beta:
                    nc.vector.scalar_tensor_tensor(
                        out=yv, in0=ps, scalar=a_t, in1=rb_sb,
                        op0=OP.mult, op1=OP.add,
                    )
                elif i % 2 == 0:
                    nc.vector.tensor_scalar(
                        out=yv, in0=ps, scalar1=a_t, scalar2=None, op0=OP.mult
                    )
                else:
                    nc.scalar.mul(out=yv, in_=ps, mul=a_t)

            # ---- main loop ----
            fronts = [front(j) for j in range(min(LOOK, NTILES))]
            for i in range(NTILES):
                g, il = divmod(i, GT)
                if il == 0 and g + 1 < NG:
                    issue_xg(g + 1)
                    issue_xr(g + 1)
                if i + LOOK < NTILES:
                    fronts.append(front(i + LOOK))
                back(i, *fronts.pop(0))
                if g == NG - 1:
                    # final group: drain early tiles in pairs, last 4 per tile,
                    # alternating rings (shortest possible tail)
                    if il in (1, 3):
                        eng = nc.gpsimd if il == 1 else nc.sync
                        eng.dma_start(
                            out=y_view[:, i - 1 : i + 1, :],
                            in_=y_tiles[g][:, il - 1 : il + 1, :],
                        )
                    elif il >= 4:
                        eng = nc.gpsimd if il % 2 == 0 else nc.sync
                        eng.dma_start(
                            out=y_view[:, i : i + 1, :],
                            in_=y_tiles[g][:, il : il + 1, :],
                        )
                elif il == GT - 1:
                    # mid-kernel y drains ride the otherwise-idle gpsimd ring
                    nc.gpsimd.dma_start(
                        out=y_view[:, g * GT : (g + 1) * GT, :], in_=y_tiles[g]
                    )

    nc.compile()
    return nc


def _get_nc(kbf, apply_beta):
    key = (kbf, apply_beta)
    if key not in _NC_CACHE:
        _NC_CACHE[key] = _build_mixed(kbf, apply_beta)
    return _NC_CACHE[key]


def _prep(x, w, g, lb, kbf, apply_beta):
    kb8 = KB - kbf
    xf = np.ascontiguousarray(x.reshape(B * S, D))
    xb = xf.astype(BF16)
    xbt = np.ascontiguousarray(xb[:, : kbf * P].T)
    if kb8:
        x8 = xf[:, kbf * P :].astype(FP8)
        x8t = np.ascontiguousarray(x8.T)

    beta = float(np.mean(np.abs(w), dtype=np.float32))
    wq = np.clip(np.round(w / np.float32(beta + EPS)), -1.0, 1.0)
    wqe = wq * g[:, None]  # ln_gamma folded (identity for the fast path)
    wqb = wqe[: kbf * P].astype(BF16)
    if kb8:
        wq8 = wqe[kbf * P :].astype(FP8)
    cs = np.ascontiguousarray(
        np.broadcast_to(
            wqb.astype(np.float32)[..., :].sum(axis=0, dtype=np.float32)
            + (
                wq8.astype(np.float32).sum(axis=0, dtype=np.float32)
                if kb8
                else 0.0
            ),
            (P, U),
        )
    ).astype(BF16)
    bcol = np.full((P, 1), beta, dtype=np.float32)
    if apply_beta:
        rb = np.ascontiguousarray(
            np.broadcast_to((beta * (lb @ wq)).astype(np.float32), (P, U))
        ).astype(np.float32)

    in_maps = []
    for c in range(N_CORES):
        sl = slice(c * TOK, (c + 1) * TOK)
        m = {
            "xbt": np.ascontiguousarray(xbt[:, sl]),
            "xr": np.ascontiguousarray(xb[sl]),
            "wqb": wqb,
            "cs": cs,
            "bcol": bcol,
        }
        if kb8:
            m["x8t"] = np.ascontiguousarray(x8t[:, sl])
            m["wq8"] = wq8
        if apply_beta:
            m["rb"] = rb
        in_maps.append(m)
    return in_maps


def run(inputs, trace=False, tmpdir=None):
    """Shard, run on 8 cores, gather. Returns (y, BassKernelResults)."""
    from concourse.bass_utils import run_bass_kernel_spmd

    x = np.asarray(inputs["x"], dtype=np.float32)
    w = np.ascontiguousarray(np.asarray(inputs["weight"], dtype=np.float32))
    g = np.ascontiguousarray(np.asarray(inputs["ln_gamma"], dtype=np.float32))
    lb = np.ascontiguousarray(np.asarray(inputs["ln_beta"], dtype=np.float32))

    # fp8 blocks are only exact for the unscaled ternary weights; with a
    # non-trivial ln_gamma fold, run all-bf16 instead (still ~2.9e-3).
    kbf = KBF if bool(np.all(g == 1.0)) else KB
    apply_beta = not bool(np.all(lb == 0.0))

    nc = _get_nc(kbf, apply_beta)
    in_maps = _prep(x, w, g, lb, kbf, apply_beta)
    res = run_bass_kernel_spmd(
        nc, in_maps, core_ids=list(range(N_CORES)), trace=trace, tmpdir=tmpdir
    )
    y = np.concatenate([r["y"].astype(np.float32) for r in res.results], axis=0)
    return y.reshape(B, S, U), res


def kernel(**inputs) -> np.ndarray:
    y, _ = run(inputs, trace=False)
    return y


# revision 40
# speedup vs baseline: 1.0226x; 1.0226x over previous
"""Trainium2 Bass kernel for nn_BitLinear (LayerNorm -> 1.58-bit BitLinear).

Math notes
----------
Reference computes, per the module:
    xn    = LN(x) * ln_gamma + ln_beta            (eps = 1e-3)
    beta  = mean(|W|);  w_q = clip(round(W / (beta + 1e-5)), -1, 1)
    gamma = max(|xn|)   (global absmax)
    xq    = clip(xn * 128 / gamma, -128 + 1e-5, 128 - 1e-5)
    y     = (xq @ w_q) * (gamma * beta / 128)

The gamma factor cancels exactly: (xn*128/gamma) @ w_q * (gamma*beta/128)
== (xn @ w_q) * beta.  The clip only affects elements within relative
7.8e-8 of the global absmax -- far below f32 matmul roundoff.  So the
kernel computes y = (LN(x) @ w_q) * beta, fully data-parallel over
tokens (no collectives).

LayerNorm folds into the matmul:
    LN(x) @ wq = s * (x @ wq - mu * colsum),   colsum[u] = sum_d wq[d,u]
The PE runs on RAW x shipped pre-transposed from the host (no on-device
transposes, no normalize pass).  The -mu*colsum term is PRELOADED into
PSUM before each tile's matmuls: the matmuls run with start=False and
accumulate on top (a one-time prologue "warmup" matmul per PSUM slot
sets the has_written bits so accumulate mode stays armed; engine writes
overwrite values but don't clear the bits).  The epilogue is a single
per-partition scale y = ps * (s*beta), alternating DVE/ACT per tile.

Precision/throughput split (measured on HW: bf16 K=128 matmul ~230ns,
fp8 DoubleRow K=256 matmul ~259ns -- 1.97x per unit contraction):
4 of 8 k-blocks run in bf16, the other 4 as two fp8(e4m3) DoubleRow
pairs per 512-wide half.  The fp8 quantization noise on half the
contraction costs rel-err 1.888e-2 vs the 2e-2 gate (measured on HW,
bit-matching a numpy simulation of the same scheme; the statistic is
an average over 33M outputs, so it is insensitive to the input draw).
The ternary w_q is exact in both dtypes.  Stats (mean/var) come from a
bf16 row-layout copy of x.

Host prep (one-time, tiny vs the 128 MB activation tensor): ternarize W
(beta = mean|W| "computed once" per the sharding hint), colsum, dtype
casts + transpose.  All O(tokens) math stays on device.

Sharding: data-parallel over the 32768 tokens, 4096 per core; weight
replicated.  If ln_gamma/ln_beta are non-trivial (not the case for the
reference inputs), a slower all-bf16 variant of the same program folds
gamma into the weights and adds beta*(ln_beta @ w_q) in the epilogue.

Scheduling notes (from perfetto traces): DMA rings are per-engine FIFO
and a dma_start can block its issuing engine's queue until ring slots
free, so the scalar (ACT) engine -- which runs sqrt/preload/epilogues
-- only issues DMAs whose data is needed late; a dummy sqrt leads the
scalar queue to hoist the one-time 1.3us ACT_TABLE_LOAD off the
critical path; mid-kernel y drains ride the otherwise-idle gpsimd
(SWDGE) ring; the PSUM-preload chain (stats -> -mu -> preload) never
depends on the sqrt path so PSUM recycling can't stall on the ACT
queue; the first four preloads run on DVE while the scalar queue is
still draining prologue dma_starts.
"""

import numpy as np
import ml_dtypes

B, S, D, U = 4, 8192, 1024, 1024
N_CORES = 8
TOK = (B * S) // N_CORES  # 4096 tokens per core
P = 128
KB = D // P               # 8 contraction blocks
KBF = 4                   # k-blocks in bf16; the last KB-KBF run in fp8
NTILES = TOK // P         # 32 token tiles per core
GT = 8                    # token tiles per DMA group
NG = NTILES // GT         # 4 groups
LOOK = 3                  # front-runs stats/preload this many tiles ahead
NPS = 4                   # PSUM slots (2 banks each)
LN_EPS = 1e-3
EPS = 1e-5

BF16 = ml_dtypes.bfloat16
FP8 = ml_dtypes.float8_e4m3fn

_NC_CACHE = {}


def _build_mixed(kbf=KBF, apply_beta=False):
    """bf16 + fp8-DoubleRow kernel (kbf=8 => all-bf16 general variant)."""
    import concourse.bacc as bacc
    import concourse.mybir as mybir
    import concourse.tile as tile
    from concourse.bass import ts

    fp32 = mybir.dt.float32
    bf16 = mybir.dt.bfloat16
    fp8 = mybir.dt.float8e4
    AF = mybir.ActivationFunctionType
    OP = mybir.AluOpType
    DR = mybir.MatmulPerfMode.DoubleRow
    kb8 = KB - kbf

    nc = bacc.Bacc()
    xb_h = nc.dram_tensor("xbt", [kbf * P, TOK], bf16, kind="ExternalInput")
    x8_h = (
        nc.dram_tensor("x8t", [kb8 * P, TOK], fp8, kind="ExternalInput")
        if kb8
        else None
    )
    xr_h = nc.dram_tensor("xr", [TOK, D], bf16, kind="ExternalInput")
    wb_h = nc.dram_tensor("wqb", [kbf * P, U], bf16, kind="ExternalInput")
    w8_h = (
        nc.dram_tensor("wq8", [kb8 * P, U], fp8, kind="ExternalInput")
        if kb8
        else None
    )
    cs_h = nc.dram_tensor("cs", [P, U], bf16, kind="ExternalInput")
    bc_h = nc.dram_tensor("bcol", [P, 1], fp32, kind="ExternalInput")
    rb_h = (
        nc.dram_tensor("rb", [P, U], fp32, kind="ExternalInput")
        if apply_beta
        else None
    )
    y_h = nc.dram_tensor("y", [TOK, U], bf16, kind="ExternalOutput")

    xr_view = xr_h[:, :].rearrange("(o p) d -> p o d", p=P)    # [128, 32, 1024]
    xb_view = xb_h[:, :].rearrange("(k q) t -> q k t", q=P)    # [128, kbf, 4096]
    x8_view = x8_h[:, :].rearrange("(k q) t -> q k t", q=P) if kb8 else None
    wb_view = wb_h[:, :].rearrange("(k q) u -> q k u", q=P)    # [128, kbf, 1024]
    w8_view = w8_h[:, :].rearrange("(k q) u -> q k u", q=P) if kb8 else None
    y_view = y_h[:, :].rearrange("(o p) u -> p o u", p=P)      # [128, 32, 1024]

    with tile.TileContext(nc) as tc:
        with (
            tc.tile_pool(name="singles", bufs=1) as singles,
            tc.tile_pool(name="xrg", bufs=4) as xrg_pool,
            tc.tile_pool(name="xbg", bufs=3) as xbg_pool,
            tc.tile_pool(name="x8g", bufs=3) as x8g_pool,
            tc.tile_pool(name="yg", bufs=2) as yg_pool,
            tc.tile_pool(name="stats", bufs=2 + LOOK) as stats_pool,
            tc.tile_pool(name="ps_y", bufs=NPS, space="PSUM") as ps_pool,
        ):
            xr_tiles = [None] * NG
            xb_tiles = [None] * NG
            x8_tiles = [None] * NG
            y_tiles = [None] * NG

            def issue_xg(g):
                tb = xbg_pool.tile([P, kbf, GT * P], bf16, tag="xb", name=f"xb{g}")
                base = g * GT * P
                for c in range(2):
                    nc.sync.dma_start(
                        out=tb[:, :, c * 512 : (c + 1) * 512],
                        in_=xb_view[:, :, base + c * 512 : base + (c + 1) * 512],
                    )
                xb_tiles[g] = tb
                if kb8:
                    t8 = x8g_pool.tile(
                        [P, kb8, GT * P], fp8, tag="x8", name=f"x8{g}"
                    )
                    nc.sync.dma_start(
                        out=t8, in_=x8_view[:, :, base : base + GT * P]
                    )
                    x8_tiles[g] = t8

            def issue_xr(g):
                t = xrg_pool.tile([P, GT, D], bf16, tag="xr", name=f"xr{g}")
                nc.scalar.dma_start(out=t, in_=xr_view[:, g * GT : (g + 1) * GT, :])
                xr_tiles[g] = t

            # ---- prologue: group-0 + weights arrive in dependency order ----
            # tile-0 critical chain: {cs,bc,xr c0} -> stats -> preload, plus
            # {xb0/x80 first chunk, wb k0} -> first matmuls; everything else
            # streams behind in per-kb / per-2-tile chunks.
            # Dummy sqrt FIRST on the scalar queue: pulls the 1.3us
            # ACT_TABLE_LOAD to t~3us, before any dma_start can block it.
            warm_s = singles.tile([P, 1], fp32)
            nc.vector.memset(warm_s, 1.0)
            nc.scalar.activation(
                out=warm_s, in_=warm_s, func=AF.Sqrt, bias=0.0, scale=1.0
            )
            wb_sb = singles.tile([P, kbf, U], bf16)
            w8_sb = singles.tile([P, kb8, U], fp8, name="w8_sb") if kb8 else None
            cs_sb = singles.tile([P, U], bf16)
            nc.sync.dma_start(out=cs_sb, in_=cs_h[:, :])
            bc_sb = singles.tile([P, 1], fp32)
            nc.sync.dma_start(out=bc_sb, in_=bc_h[:, :])
            if apply_beta:
                rb_sb = singles.tile([P, U], fp32)
                nc.sync.dma_start(out=rb_sb, in_=rb_h[:, :])
            xr0 = xrg_pool.tile([P, GT, D], bf16, tag="xr", name="xr0")
            xr_tiles[0] = xr0
            nc.scalar.dma_start(out=xr0[:, 0:1, :], in_=xr_view[:, 0:1, :])
            nc.scalar.dma_start(out=xr0[:, 1:2, :], in_=xr_view[:, 1:2, :])
            xb0 = xbg_pool.tile([P, kbf, GT * P], bf16, tag="xb", name="xb0")
            xb_tiles[0] = xb0
            nc.sync.dma_start(out=xb0[:, :, 0:256], in_=xb_view[:, :, 0:256])
            if kb8:
                x80 = x8g_pool.tile([P, kb8, GT * P], fp8, tag="x8", name="x80")
                x8_tiles[0] = x80
                nc.sync.dma_start(out=x80[:, :, 0:256], in_=x8_view[:, :, 0:256])
            nc.scalar.dma_start(out=wb_sb[:, 0, :], in_=wb_view[:, 0, :])
            nc.scalar.dma_start(out=wb_sb[:, 1, :], in_=wb_view[:, 1, :])
            nc.scalar.dma_start(out=xr0[:, 2:4, :], in_=xr_view[:, 2:4, :])
            nc.sync.dma_start(out=wb_sb[:, 2:kbf, :], in_=wb_view[:, 2:kbf, :])
            if kb8:
                nc.sync.dma_start(out=w8_sb, in_=w8_view[:, :, :])
            nc.sync.dma_start(out=xb0[:, :, 256:512], in_=xb_view[:, :, 256:512])
            if kb8:
                nc.sync.dma_start(
                    out=x80[:, :, 256:512], in_=x8_view[:, :, 256:512]
                )
            nc.scalar.dma_start(out=xr0[:, 4:GT, :], in_=xr_view[:, 4:GT, :])
            nc.sync.dma_start(out=xb0[:, :, 512:768], in_=xb_view[:, :, 512:768])
            if kb8:
                nc.sync.dma_start(
                    out=x80[:, :, 512:1024], in_=x8_view[:, :, 512:1024]
                )
            nc.sync.dma_start(out=xb0[:, :, 768:1024], in_=xb_view[:, :, 768:1024])

            eps_t = singles.tile([P, 1], fp32)
            nc.vector.memset(eps_t, LN_EPS)

            # ---- PSUM warmup: one start=True matmul per slot half sets the
            # has_written bits so all later matmuls can run start=False and
            # accumulate on top of the preloaded -mu*colsum values. ----
            z_l = singles.tile([1, P], bf16)
            nc.vector.memset(z_l, 0.0)
            z_r = singles.tile([1, U], bf16)
            nc.vector.memset(z_r, 0.0)
            for sl in range(NPS):
                ps = ps_pool.tile([P, U], fp32, tag="ps", name=f"warm{sl}")
                for h in range(2):
                    nc.tensor.matmul(
                        ps[:, ts(h, 512)], lhsT=z_l, rhs=z_r[:, ts(h, 512)],
                        start=True, stop=True,
                    )

            # ---- per-tile pieces ----
            def front(i):
                """Stats chain + PSUM preload; runs LOOK tiles ahead of PE.

                The preload chain (aggr -> nm -> preload) never touches the
                sqrt path, so a slow ACT queue can't stall PSUM recycling;
                recip/a (epilogue scale) are deferred to back().
                """
                g, il = divmod(i, GT)
                xv = xr_tiles[g][:, il, :]
                xvr = xv.rearrange("p (n f) -> p n f", f=512)
                st = stats_pool.tile([P, 2, 6], fp32, tag="st")
                nc.vector.bn_stats(out=st[:, 0, :], in_=xvr[:, 0, :])
                nc.vector.bn_stats(out=st[:, 1, :], in_=xvr[:, 1, :])
                mv = stats_pool.tile([P, 2], fp32, tag="mv")
                nc.vector.bn_aggr(out=mv, in_=st)
                nm = stats_pool.tile([P, 1], fp32, tag="nm")
                nc.vector.tensor_scalar(
                    out=nm, in0=mv[:, 0:1], scalar1=-1.0, scalar2=None, op0=OP.mult
                )
                # preload: ps <- cs * (-mu)   (overwrite; has_written bits
                # stay set).  Tiles 0-3 use DVE: the scalar queue is still
                # draining its prologue dma_starts at that point.
                ps = ps_pool.tile([P, U], fp32, tag="ps")
                if i < 4:
                    nc.vector.tensor_scalar(
                        out=ps, in0=cs_sb, scalar1=nm, scalar2=None, op0=OP.mult
                    )
                else:
                    nc.scalar.mul(out=ps, in_=cs_sb, mul=nm)
                sq = stats_pool.tile([P, 1], fp32, tag="sq")
                nc.scalar.activation(
                    out=sq, in_=mv[:, 1:2], func=AF.Sqrt, bias=eps_t, scale=1.0
                )
                return sq, ps

            def back(i, sq, ps):
                """bf16 + fp8-DoubleRow matmul sweep + scale epilogue."""
                g, il = divmod(i, GT)
                s_t = stats_pool.tile([P, 1], fp32, tag="s")
                nc.vector.reciprocal(s_t, sq)
                a_t = stats_pool.tile([P, 1], fp32, tag="a")
                nc.vector.tensor_tensor(a_t, s_t, bc_sb, OP.mult)
                if il == 0:
                    y_tiles[g] = yg_pool.tile([P, GT, U], bf16, tag="y", name=f"y{g}")
                lbt, l8 = xb_tiles[g], x8_tiles[g]
                tok = slice(il * P, (il + 1) * P)
                npair = kb8 // 2
                for h in range(2):
                    for k in range(kbf):
                        nc.tensor.matmul(
                            ps[:, ts(h, 512)], lhsT=lbt[:, k, tok],
                            rhs=wb_sb[:, k, ts(h, 512)],
                            start=False, stop=(npair == 0 and k == kbf - 1),
                        )
                    for j in range(npair):
                        pr = slice(2 * j, 2 * j + 2)
                        nc.tensor.matmul(
                            ps[:, ts(h, 512)], lhsT=l8[:, pr, tok],
                            rhs=w8_sb[:, pr, ts(h, 512)],
                            start=False, stop=(j == npair - 1), perf_mode=DR,
                        )
                # epilogue: y = ps * a (+ rb); alternate engines in the fast
                # path so neither queue saturates
                yv = y_tiles[g][:, il, :]
                if apply_beta:
                    nc.vector.scalar_tensor_tensor(
                        out=yv, in0=ps, scalar=a_t, in1=rb_sb,
                        op0=OP.mult, op1=OP.add,
                    )
                elif i % 2 == 0:
                    nc.vector.tensor_scalar(
                        out=yv, in0=ps, scalar1=a_t, scalar2=None, op0=OP.mult
                    )
                else:
                    nc.scalar.mul(out=yv, in_=ps, mul=a_t)

            # ---- main loop ----
            fronts = [front(j) for j in range(min(LOOK, NTILES))]
            for i in range(NTILES):
                g, il = divmod(i, GT)
                if il == 0 and g + 1 < NG:
                    issue_xg(g + 1)
                    issue_xr(g + 1)
                if i + LOOK < NTILES:
                    fronts.append(front(i + LOOK))
                back(i, *fronts.pop(0))
                if g == NG - 1:
                    # final group: drain early tiles in pairs, last 4 per tile,
                    # alternating rings (shortest possible tail)
                    if il in (1, 3):
                        eng = nc.gpsimd if il == 1 else nc.sync
                        eng.dma_start(
                            out=y_view[:, i - 1 : i + 1, :],
                            in_=y_tiles[g][:, il - 1 : il + 1, :],
                        )
                    elif il >= 4:
                        eng = nc.gpsimd if il % 2 == 0 else nc.sync
                        eng.dma_start(
                            out=y_view[:, i : i + 1, :],
                            in_=y_tiles[g][:, il : il + 1, :],
                        )
                elif il == GT - 1:
                    # mid-kernel y drains ride the otherwise-idle gpsimd ring
                    nc.gpsimd.dma_start(
                        out=y_view[:, g * GT : (g + 1) * GT, :], in_=y_tiles[g]
                    )

    nc.compile()
    return nc


def _get_nc(kbf, apply_beta):
    key = (kbf, apply_beta)
    if key not in _NC_CACHE:
        _NC_CACHE[key] = _build_mixed(kbf, apply_beta)
    return _NC_CACHE[key]


def _prep(x, w, g, lb, kbf, apply_beta):
    kb8 = KB - kbf
    xf = np.ascontiguousarray(x.reshape(B * S, D))
    xb = xf.astype(BF16)
    xbt = np.ascontiguousarray(xb[:, : kbf * P].T)
    if kb8:
        x8 = xf[:, kbf * P :].astype(FP8)
        x8t = np.ascontiguousarray(x8.T)

    beta = float(np.mean(np.abs(w), dtype=np.float32))
    wq = np.clip(np.round(w / np.float32(beta + EPS)), -1.0, 1.0)
    wqe = wq * g[:, None]  # ln_gamma folded (identity for the fast path)
    wqb = wqe[: kbf * P].astype(BF16)
    if kb8:
        wq8 = wqe[kbf * P :].astype(FP8)
    cs = np.ascontiguousarray(
        np.broadcast_to(
            wqb.astype(np.float32)[..., :].sum(axis=0, dtype=np.float32)
            + (
                wq8.astype(np.float32).sum(axis=0, dtype=np.float32)
                if kb8
                else 0.0
            ),
            (P, U),
        )
    ).astype(BF16)
    bcol = np.full((P, 1), beta, dtype=np.float32)
    if apply_beta:
        rb = np.ascontiguousarray(
            np.broadcast_to((beta * (lb @ wq)).astype(np.float32), (P, U))
        ).astype(np.float32)

    in_maps = []
    for c in range(N_CORES):
        sl = slice(c * TOK, (c + 1) * TOK)
        m = {
            "xbt": np.ascontiguousarray(xbt[:, sl]),
            "xr": np.ascontiguousarray(xb[sl]),
            "wqb": wqb,
            "cs": cs,
            "bcol": bcol,
        }
        if kb8:
            m["x8t"] = np.ascontiguousarray(x8t[:, sl])
            m["wq8"] = wq8
        if apply_beta:
            m["rb"] = rb
        in_maps.append(m)
    return in_maps


def run(inputs, trace=False, tmpdir=None):
    """Shard, run on 8 cores, gather. Returns (y, BassKernelResults)."""
    from concourse.bass_utils import run_bass_kernel_spmd

    x = np.asarray(inputs["x"], dtype=np.float32)
    w = np.ascontiguousarray(np.asarray(inputs["weight"], dtype=np.float32))
    g = np.ascontiguousarray(np.asarray(inputs["ln_gamma"], dtype=np.float32))
    lb = np.ascontiguousarray(np.asarray(inputs["ln_beta"], dtype=np.float32))

    # fp8 blocks are only exact for the unscaled ternary weights; with a
    # non-trivial ln_gamma fold, run all-bf16 instead (still ~2.9e-3).
    kbf = KBF if bool(np.all(g == 1.0)) else KB
    apply_beta = not bool(np.all(lb == 0.0))

    nc = _get_nc(kbf, apply_beta)
    in_maps = _prep(x, w, g, lb, kbf, apply_beta)
    res = run_bass_kernel_spmd(
        nc, in_maps, core_ids=list(range(N_CORES)), trace=trace, tmpdir=tmpdir
    )
    y = np.concatenate([r["y"].astype(np.float32) for r in res.results], axis=0)
    return y.reshape(B, S, U), res


def kernel(**inputs) -> np.ndarray:
    y, _ = run(inputs, trace=False)
    return y


# revision 41
# speedup vs baseline: 1.0322x; 1.0094x over previous
"""Trainium2 Bass kernel for nn_BitLinear (LayerNorm -> 1.58-bit BitLinear).

Math notes
----------
Reference computes, per the module:
    xn    = LN(x) * ln_gamma + ln_beta            (eps = 1e-3)
    beta  = mean(|W|);  w_q = clip(round(W / (beta + 1e-5)), -1, 1)
    gamma = max(|xn|)   (global absmax)
    xq    = clip(xn * 128 / gamma, -128 + 1e-5, 128 - 1e-5)
    y     = (xq @ w_q) * (gamma * beta / 128)

The gamma factor cancels exactly: (xn*128/gamma) @ w_q * (gamma*beta/128)
== (xn @ w_q) * beta.  The clip only affects elements within relative
7.8e-8 of the global absmax -- far below f32 matmul roundoff.  So the
kernel computes y = (LN(x) @ w_q) * beta, fully data-parallel over
tokens (no collectives).

LayerNorm folds into the matmul:
    LN(x) @ wq = s * (x @ wq - mu * colsum),   colsum[u] = sum_d wq[d,u]
The PE runs on RAW x shipped pre-transposed from the host (no on-device
transposes, no normalize pass).  The -mu*colsum term is PRELOADED into
PSUM before each tile's matmuls: the matmuls run with start=False and
accumulate on top (a one-time prologue "warmup" matmul per PSUM slot
sets the has_written bits so accumulate mode stays armed; engine writes
overwrite values but don't clear the bits).  The epilogue is a single
per-partition scale y = ps * (s*beta), alternating DVE/ACT per tile.

Precision/throughput split (measured on HW: bf16 K=128 matmul ~230ns,
fp8 DoubleRow K=256 matmul ~259ns -- 1.97x per unit contraction):
4 of 8 k-blocks run in bf16, the other 4 as two fp8(e4m3) DoubleRow
pairs per 512-wide half.  The fp8 quantization noise on half the
contraction costs rel-err 1.888e-2 vs the 2e-2 gate (measured on HW,
bit-matching a numpy simulation of the same scheme; the statistic is
an average over 33M outputs, so it is insensitive to the input draw).
The ternary w_q is exact in both dtypes.  Stats (mean/var) come from a
bf16 row-layout copy of x.

Host prep (one-time, tiny vs the 128 MB activation tensor): ternarize W
(beta = mean|W| "computed once" per the sharding hint), colsum, dtype
casts + transpose.  All O(tokens) math stays on device.

Sharding: data-parallel over the 32768 tokens, 4096 per core; weight
replicated.  If ln_gamma/ln_beta are non-trivial (not the case for the
reference inputs), a slower all-bf16 variant of the same program folds
gamma into the weights and adds beta*(ln_beta @ w_q) in the epilogue.

Scheduling notes (from perfetto traces): DMA rings are per-engine FIFO
and a dma_start can block its issuing engine's queue until ring slots
free, so the scalar (ACT) engine -- which runs sqrt/preload/epilogues
-- only issues DMAs whose data is needed late; a dummy sqrt leads the
scalar queue to hoist the one-time 1.3us ACT_TABLE_LOAD off the
critical path; mid-kernel y drains ride the otherwise-idle gpsimd
(SWDGE) ring; the PSUM-preload chain (stats -> -mu -> preload) never
depends on the sqrt path so PSUM recycling can't stall on the ACT
queue; the first four preloads run on DVE while the scalar queue is
still draining prologue dma_starts.
"""

import numpy as np
import ml_dtypes

B, S, D, U = 4, 8192, 1024, 1024
N_CORES = 8
TOK = (B * S) // N_CORES  # 4096 tokens per core
P = 128
KB = D // P               # 8 contraction blocks
KBF = 4                   # k-blocks in bf16; the last KB-KBF run in fp8
NTILES = TOK // P         # 32 token tiles per core
GT = 8                    # token tiles per DMA group
NG = NTILES // GT         # 4 groups
LOOK = 3                  # front-runs stats/preload this many tiles ahead
NPS = 4                   # PSUM slots (2 banks each)
LN_EPS = 1e-3
EPS = 1e-5

BF16 = ml_dtypes.bfloat16
FP8 = ml_dtypes.float8_e4m3fn

_NC_CACHE = {}


def _build_mixed(kbf=KBF, apply_beta=False):
    """bf16 + fp8-DoubleRow kernel (kbf=8 => all-bf16 general variant)."""
    import concourse.bacc as bacc
    import concourse.mybir as mybir
    import concourse.tile as tile
    from concourse.bass import ts

    fp32 = mybir.dt.float32
    bf16 = mybir.dt.bfloat16
    fp8 = mybir.dt.float8e4
    AF = mybir.ActivationFunctionType
    OP = mybir.AluOpType
    DR = mybir.MatmulPerfMode.DoubleRow
    kb8 = KB - kbf

    nc = bacc.Bacc()
    xb_h = nc.dram_tensor("xbt", [kbf * P, TOK], bf16, kind="ExternalInput")
    x8_h = (
        nc.dram_tensor("x8t", [kb8 * P, TOK], fp8, kind="ExternalInput")
        if kb8
        else None
    )
    xr_h = nc.dram_tensor("xr", [TOK, D], bf16, kind="ExternalInput")
    wb_h = nc.dram_tensor("wqb", [kbf * P, U], bf16, kind="ExternalInput")
    w8_h = (
        nc.dram_tensor("wq8", [kb8 * P, U], fp8, kind="ExternalInput")
        if kb8
        else None
    )
    cs_h = nc.dram_tensor("cs", [P, U], bf16, kind="ExternalInput")
    bc_h = nc.dram_tensor("bcol", [P, 1], fp32, kind="ExternalInput")
    rb_h = (
        nc.dram_tensor("rb", [P, U], fp32, kind="ExternalInput")
        if apply_beta
        else None
    )
    y_h = nc.dram_tensor("y", [TOK, U], bf16, kind="ExternalOutput")

    xr_view = xr_h[:, :].rearrange("(o p) d -> p o d", p=P)    # [128, 32, 1024]
    xb_view = xb_h[:, :].rearrange("(k q) t -> q k t", q=P)    # [128, kbf, 4096]
    x8_view = x8_h[:, :].rearrange("(k q) t -> q k t", q=P) if kb8 else None
    wb_view = wb_h[:, :].rearrange("(k q) u -> q k u", q=P)    # [128, kbf, 1024]
    w8_view = w8_h[:, :].rearrange("(k q) u -> q k u", q=P) if kb8 else None
    y_view = y_h[:, :].rearrange("(o p) u -> p o u", p=P)      # [128, 32, 1024]

    with tile.TileContext(nc) as tc:
        with (
            tc.tile_pool(name="singles", bufs=1) as singles,
            tc.tile_pool(name="xrg", bufs=4) as xrg_pool,
            tc.tile_pool(name="xbg", bufs=3) as xbg_pool,
            tc.tile_pool(name="x8g", bufs=3) as x8g_pool,
            tc.tile_pool(name="yg", bufs=2) as yg_pool,
            tc.tile_pool(name="stats", bufs=2 + LOOK) as stats_pool,
            tc.tile_pool(name="ps_y", bufs=NPS, space="PSUM") as ps_pool,
        ):
            xr_tiles = [None] * NG
            xb_tiles = [None] * NG
            x8_tiles = [None] * NG
            y_tiles = [None] * NG

            def issue_xg(g):
                tb = xbg_pool.tile([P, kbf, GT * P], bf16, tag="xb", name=f"xb{g}")
                base = g * GT * P
                for c in range(2):
                    nc.sync.dma_start(
                        out=tb[:, :, c * 512 : (c + 1) * 512],
                        in_=xb_view[:, :, base + c * 512 : base + (c + 1) * 512],
                    )
                xb_tiles[g] = tb
                if kb8:
                    t8 = x8g_pool.tile(
                        [P, kb8, GT * P], fp8, tag="x8", name=f"x8{g}"
                    )
                    nc.sync.dma_start(
                        out=t8, in_=x8_view[:, :, base : base + GT * P]
                    )
                    x8_tiles[g] = t8

            def issue_xr(g):
                t = xrg_pool.tile([P, GT, D], bf16, tag="xr", name=f"xr{g}")
                nc.scalar.dma_start(out=t, in_=xr_view[:, g * GT : (g + 1) * GT, :])
                xr_tiles[g] = t

            # ---- prologue: group-0 + weights arrive in dependency order ----
            # tile-0 critical chain: {cs,bc,xr c0} -> stats -> preload, plus
            # {xb0/x80 first chunk, wb k0} -> first matmuls; everything else
            # streams behind in per-kb / per-2-tile chunks.
            # Dummy sqrt FIRST on the scalar queue: pulls the 1.3us
            # ACT_TABLE_LOAD to t~3us, before any dma_start can block it.
            warm_s = singles.tile([P, 1], fp32)
            nc.vector.memset(warm_s, 1.0)
            nc.scalar.activation(
                out=warm_s, in_=warm_s, func=AF.Sqrt, bias=0.0, scale=1.0
            )
            wb_sb = singles.tile([P, kbf, U], bf16)
            w8_sb = singles.tile([P, kb8, U], fp8, name="w8_sb") if kb8 else None
            cs_sb = singles.tile([P, U], bf16)
            nc.sync.dma_start(out=cs_sb, in_=cs_h[:, :])
            bc_sb = singles.tile([P, 1], fp32)
            nc.sync.dma_start(out=bc_sb, in_=bc_h[:, :])
            if apply_beta:
                rb_sb = singles.tile([P, U], fp32)
                nc.sync.dma_start(out=rb_sb, in_=rb_h[:, :])
            xr0 = xrg_pool.tile([P, GT, D], bf16, tag="xr", name="xr0")
            xr_tiles[0] = xr0
            nc.scalar.dma_start(out=xr0[:, 0:1, :], in_=xr_view[:, 0:1, :])
            nc.scalar.dma_start(out=xr0[:, 1:2, :], in_=xr_view[:, 1:2, :])
            xb0 = xbg_pool.tile([P, kbf, GT * P], bf16, tag="xb", name="xb0")
            xb_tiles[0] = xb0
            nc.sync.dma_start(out=xb0[:, :, 0:256], in_=xb_view[:, :, 0:256])
            if kb8:
                x80 = x8g_pool.tile([P, kb8, GT * P], fp8, tag="x8", name="x80")
                x8_tiles[0] = x80
                nc.sync.dma_start(out=x80[:, :, 0:256], in_=x8_view[:, :, 0:256])
            nc.scalar.dma_start(out=wb_sb[:, 0, :], in_=wb_view[:, 0, :])
            nc.scalar.dma_start(out=wb_sb[:, 1, :], in_=wb_view[:, 1, :])
            nc.scalar.dma_start(out=xr0[:, 2:4, :], in_=xr_view[:, 2:4, :])
            nc.sync.dma_start(out=wb_sb[:, 2:kbf, :], in_=wb_view[:, 2:kbf, :])
            if kb8:
                nc.sync.dma_start(out=w8_sb, in_=w8_view[:, :, :])
            nc.sync.dma_start(out=xb0[:, :, 256:512], in_=xb_view[:, :, 256:512])
            if kb8:
                nc.sync.dma_start(
                    out=x80[:, :, 256:512], in_=x8_view[:, :, 256:512]
                )
            nc.scalar.dma_start(out=xr0[:, 4:GT, :], in_=xr_view[:, 4:GT, :])
            nc.sync.dma_start(out=xb0[:, :, 512:768], in_=xb_view[:, :, 512:768])
            if kb8:
                nc.sync.dma_start(
                    out=x80[:, :, 512:1024], in_=x8_view[:, :, 512:1024]
                )
            nc.sync.dma_start(out=xb0[:, :, 768:1024], in_=xb_view[:, :, 768:1024])

            eps_t = singles.tile([P, 1], fp32)
            nc.vector.memset(eps_t, LN_EPS)

            # ---- PSUM warmup: one start=True matmul per slot half sets the
            # has_written bits so all later matmuls can run start=False and
            # accumulate on top of the preloaded -mu*colsum values. ----
            z_l = singles.tile([1, P], bf16)
            nc.vector.memset(z_l, 0.0)
            z_r = singles.tile([1, U], bf16)
            nc.vector.memset(z_r, 0.0)
            for sl in range(NPS):
                ps = ps_pool.tile([P, U], fp32, tag="ps", name=f"warm{sl}")
                for h in range(2):
                    nc.tensor.matmul(
                        ps[:, ts(h, 512)], lhsT=z_l, rhs=z_r[:, ts(h, 512)],
                        start=True, stop=True,
                    )

            # ---- per-tile pieces ----
            def front(i):
                """Stats chain + PSUM preload; runs LOOK tiles ahead of PE.

                The preload chain (aggr -> nm -> preload) never touches the
                sqrt path, so a slow ACT queue can't stall PSUM recycling;
                recip/a (epilogue scale) are deferred to back().
                """
                g, il = divmod(i, GT)
                xv = xr_tiles[g][:, il, :]
                xvr = xv.rearrange("p (n f) -> p n f", f=512)
                st = stats_pool.tile([P, 2, 6], fp32, tag="st")
                nc.vector.bn_stats(out=st[:, 0, :], in_=xvr[:, 0, :])
                nc.vector.bn_stats(out=st[:, 1, :], in_=xvr[:, 1, :])
                mv = stats_pool.tile([P, 2], fp32, tag="mv")
                nc.vector.bn_aggr(out=mv, in_=st)
                nm = stats_pool.tile([P, 1], fp32, tag="nm")
                nc.vector.tensor_scalar(
                    out=nm, in0=mv[:, 0:1], scalar1=-1.0, scalar2=None, op0=OP.mult
                )
                # preload: ps <- cs * (-mu)   (overwrite; has_written bits
                # stay set).  Tiles 0-3 use DVE: the scalar queue is still
                # draining its prologue dma_starts at that point.
                ps = ps_pool.tile([P, U], fp32, tag="ps")
                if i < 4:
                    for h in range(2):
                        nc.vector.tensor_scalar(
                            out=ps[:, h * 512 : (h + 1) * 512],
                            in0=cs_sb[:, h * 512 : (h + 1) * 512],
                            scalar1=nm, scalar2=None, op0=OP.mult,
                        )
                else:
                    nc.scalar.mul(out=ps, in_=cs_sb, mul=nm)
                sq = stats_pool.tile([P, 1], fp32, tag="sq")
                nc.scalar.activation(
                    out=sq, in_=mv[:, 1:2], func=AF.Sqrt, bias=eps_t, scale=1.0
                )
                return sq, ps

            def back(i, sq, ps):
                """bf16 + fp8-DoubleRow matmul sweep + scale epilogue."""
                g, il = divmod(i, GT)
                s_t = stats_pool.tile([P, 1], fp32, tag="s")
                nc.vector.reciprocal(s_t, sq)
                a_t = stats_pool.tile([P, 1], fp32, tag="a")
                nc.vector.tensor_tensor(a_t, s_t, bc_sb, OP.mult)
                if il == 0:
                    y_tiles[g] = yg_pool.tile([P, GT, U], bf16, tag="y", name=f"y{g}")
                lbt, l8 = xb_tiles[g], x8_tiles[g]
                tok = slice(il * P, (il + 1) * P)
                npair = kb8 // 2
                for h in range(2):
                    for k in range(kbf):
                        nc.tensor.matmul(
                            ps[:, ts(h, 512)], lhsT=lbt[:, k, tok],
                            rhs=wb_sb[:, k, ts(h, 512)],
                            start=False, stop=(npair == 0 and k == kbf - 1),
                        )
                    for j in range(npair):
                        pr = slice(2 * j, 2 * j + 2)
                        nc.tensor.matmul(
                            ps[:, ts(h, 512)], lhsT=l8[:, pr, tok],
                            rhs=w8_sb[:, pr, ts(h, 512)],
                            start=False, stop=(j == npair - 1), perf_mode=DR,
                        )
                # epilogue: y = ps * a (+ rb); alternate engines in the fast
                # path so neither queue saturates
                yv = y_tiles[g][:, il, :]
                if apply_beta:
                    nc.vector.scalar_tensor_tensor(
                        out=yv, in0=ps, scalar=a_t, in1=rb_sb,
                        op0=OP.mult, op1=OP.add,
                    )
                elif i >= NTILES - 2:
                    # tail tiles: halves in parallel on both engines, each
                    # half drained as soon as it lands
                    nc.vector.tensor_scalar(
                        out=yv[:, 0:512], in0=ps[:, 0:512], scalar1=a_t,
                        scalar2=None, op0=OP.mult,
                    )
                    nc.scalar.mul(out=yv[:, 512:U], in_=ps[:, 512:U], mul=a_t)
                    e0 = nc.sync if i % 2 == 0 else nc.gpsimd
                    e1 = nc.gpsimd if i % 2 == 0 else nc.sync
                    e0.dma_start(
                        out=y_view[:, i : i + 1, 0:512],
                        in_=y_tiles[g][:, il : il + 1, 0:512],
                    )
                    e1.dma_start(
                        out=y_view[:, i : i + 1, 512:U],
                        in_=y_tiles[g][:, il : il + 1, 512:U],
                    )
                elif i % 2 == 0:
                    nc.vector.tensor_scalar(
                        out=yv, in0=ps, scalar1=a_t, scalar2=None, op0=OP.mult
                    )
                else:
                    nc.scalar.mul(out=yv, in_=ps, mul=a_t)

            # ---- main loop ----
            fronts = [front(j) for j in range(min(LOOK, NTILES))]
            for i in range(NTILES):
                g, il = divmod(i, GT)
                if il == 0 and g + 1 < NG:
                    issue_xg(g + 1)
                    issue_xr(g + 1)
                if i + LOOK < NTILES:
                    fronts.append(front(i + LOOK))
                back(i, *fronts.pop(0))
                if g == NG - 1:
                    # final group: drain early tiles in pairs, last 4 per tile,
                    # alternating rings (shortest possible tail)
                    if il in (1, 3):
                        eng = nc.gpsimd if il == 1 else nc.sync
                        eng.dma_start(
                            out=y_view[:, i - 1 : i + 1, :],
                            in_=y_tiles[g][:, il - 1 : il + 1, :],
                        )
                    elif 4 <= il < GT - 2:
                        eng = nc.gpsimd if il % 2 == 0 else nc.sync
                        eng.dma_start(
                            out=y_view[:, i : i + 1, :],
                            in_=y_tiles[g][:, il : il + 1, :],
                        )
                elif il == GT - 1:
                    # mid-kernel y drains ride the otherwise-idle gpsimd ring
                    nc.gpsimd.dma_start(
                        out=y_view[:, g * GT : (g + 1) * GT, :], in_=y_tiles[g]
                    )

    nc.compile()
    return nc


def _get_nc(kbf, apply_beta):
    key = (kbf, apply_beta)
    if key not in _NC_CACHE:
        _NC_CACHE[key] = _build_mixed(kbf, apply_beta)
    return _NC_CACHE[key]


def _prep(x, w, g, lb, kbf, apply_beta):
    kb8 = KB - kbf
    xf = np.ascontiguousarray(x.reshape(B * S, D))
    xb = xf.astype(BF16)
    xbt = np.ascontiguousarray(xb[:, : kbf * P].T)
    if kb8:
        x8 = xf[:, kbf * P :].astype(FP8)
        x8t = np.ascontiguousarray(x8.T)

    beta = float(np.mean(np.abs(w), dtype=np.float32))
    wq = np.clip(np.round(w / np.float32(beta + EPS)), -1.0, 1.0)
    wqe = wq * g[:, None]  # ln_gamma folded (identity for the fast path)
    wqb = wqe[: kbf * P].astype(BF16)
    if kb8:
        wq8 = wqe[kbf * P :].astype(FP8)
    cs = np.ascontiguousarray(
        np.broadcast_to(
            wqb.astype(np.float32)[..., :].sum(axis=0, dtype=np.float32)
            + (
                wq8.astype(np.float32).sum(axis=0, dtype=np.float32)
                if kb8
                else 0.0
            ),
            (P, U),
        )
    ).astype(BF16)
    bcol = np.full((P, 1), beta, dtype=np.float32)
    if apply_beta:
        rb = np.ascontiguousarray(
            np.broadcast_to((beta * (lb @ wq)).astype(np.float32), (P, U))
        ).astype(np.float32)

    in_maps = []
    for c in range(N_CORES):
        sl = slice(c * TOK, (c + 1) * TOK)
        m = {
            "xbt": np.ascontiguousarray(xbt[:, sl]),
            "xr": np.ascontiguousarray(xb[sl]),
            "wqb": wqb,
            "cs": cs,
            "bcol": bcol,
        }
        if kb8:
            m["x8t"] = np.ascontiguousarray(x8t[:, sl])
            m["wq8"] = wq8
        if apply_beta:
            m["rb"] = rb
        in_maps.append(m)
    return in_maps


def run(inputs, trace=False, tmpdir=None):
    """Shard, run on 8 cores, gather. Returns (y, BassKernelResults)."""
    from concourse.bass_utils import run_bass_kernel_spmd

    x = np.asarray(inputs["x"], dtype=np.float32)
    w = np.ascontiguousarray(np.asarray(inputs["weight"], dtype=np.float32))
    g = np.ascontiguousarray(np.asarray(inputs["ln_gamma"], dtype=np.float32))
    lb = np.ascontiguousarray(np.asarray(inputs["ln_beta"], dtype=np.float32))

    # fp8 blocks are only exact for the unscaled ternary weights; with a
    # non-trivial ln_gamma fold, run all-bf16 instead (still ~2.9e-3).
    kbf = KBF if bool(np.all(g == 1.0)) else KB
    apply_beta = not bool(np.all(lb == 0.0))

    nc = _get_nc(kbf, apply_beta)
    in_maps = _prep(x, w, g, lb, kbf, apply_beta)
    res = run_bass_kernel_spmd(
        nc, in_maps, core_ids=list(range(N_CORES)), trace=trace, tmpdir=tmpdir
    )
    y = np.concatenate([r["y"].astype(np.float32) for r in res.results], axis=0)
    return y.reshape(B, S, U), res


def kernel(**inputs) -> np.ndarray:
    y, _ = run(inputs, trace=False)
    return y
